# revision 1
# baseline (speedup 1.0000x reference)
"""Trainium2 Bass kernel for DGNRNetwork (2-layer TransformerConv GNN + MLPs).

Strategy (8 NeuronCores, graph/data parallel):
  - Nodes padded to N_PAD=50176 and sharded by contiguous range: core c owns
    nodes [c*6272, (c+1)*6272), i.e. 49 blocks of 128 dst nodes per core.
  - Edges are partitioned by dst shard on host, sorted by (dst block, src
    half, src), padded so every (block, src-half) group is a whole number of
    128-edge tiles (tile counts uniform across cores -> one SPMD program).
  - Per edge-block: k||v rows and q rows are fetched with indirect DMA
    (dma_gather); per-edge attention logits are computed on the Vector
    engine; exp on Scalar engine; the segment-softmax denominator and the
    weighted sum over incoming edges are ONE accumulated TensorE matmul with
    a host-precomputed one-hot scatter matrix S (S[e, d] = edge e's dst is
    block-node d). Padding edges have all-zero S rows so they drop out.
  - Small weights are replicated; k/v row tables are exchanged with an
    AllGather between the conv layers; the tiny Q-head is computed
    redundantly and combined with a masked AllReduce.
"""

import sys

sys.path.insert(0, "/opt/trn_rl_repo")

import numpy as np
import ml_dtypes

import concourse.bacc as bacc
import concourse.bass as bass
import concourse.mybir as mybir
import concourse.tile as tile
from concourse import bass_utils, library_config

F32 = mybir.dt.float32
BF16 = mybir.dt.bfloat16
I16 = mybir.dt.int16

N_CORES = 8


class Cfg:
    def __init__(self, n_nodes=50000, nblk=49, b=64, edge_bf16=True):
        self.N = n_nodes
        self.NBLK = nblk                 # dst blocks per core
        self.SHARD = nblk * 128          # nodes per core
        self.N_PAD = 8 * self.SHARD
        self.HALF = self.N_PAD // 2      # kv table split (int16 gather idx)
        self.B = b                       # batch (selected nodes)
        self.F_IN = 128
        self.H = 32
        self.HD = 128
        self.HEADS = 4
        self.EPS = 1e-16
        self.SCALE = 1.0 / np.sqrt(32.0)
        self.edge_bf16 = edge_bf16
        self.EDT = BF16 if edge_bf16 else F32
        self.EDT_NP = ml_dtypes.bfloat16 if edge_bf16 else np.float32
        assert self.N <= self.N_PAD and self.HALF < 32768


# --------------------------------------------------------------------------
# host-side preprocessing
# --------------------------------------------------------------------------


def _wrap16(values, slots):
    """dma_gather idx layout: idx i lives at [i % 16, i // 16], replicated
    across the eight 16-partition groups."""
    arr = np.zeros((16, slots // 16), dtype=np.int16)
    arr[np.arange(len(values)) % 16, np.arange(len(values)) // 16] = values
    return np.tile(arr, (8, 1))


def _prep_edges(cfg, edge_index):
    src = np.ascontiguousarray(edge_index[0]).astype(np.int64)
    dst = np.ascontiguousarray(edge_index[1]).astype(np.int64)
    core = dst // cfg.SHARD
    blk = (dst % cfg.SHARD) // 128
    hi = (src >= cfg.HALF).astype(np.int64)

    cnt = np.zeros((N_CORES, cfg.NBLK, 2), np.int64)
    np.add.at(cnt, (core, blk, hi), 1)
    t_lo = np.maximum(1, (cnt[:, :, 0].max(0) + 127) // 128)  # [NBLK]
    t_hi = np.maximum(1, (cnt[:, :, 1].max(0) + 127) // 128)
    t_all = t_lo + t_hi
    blk_off = np.zeros(cfg.NBLK + 1, np.int64)
    blk_off[1:] = np.cumsum(t_all * 128)
    slots = int(blk_off[-1])
    assert slots % 16 == 0

    order = np.lexsort((src, hi, blk, core))
    s_src, s_dst, s_core, s_blk, s_hi = (
        src[order], dst[order], core[order], blk[order], hi[order])

    per_core = []
    for c in range(N_CORES):
        m = s_core == c
        csrc, cdst, cblk, chi = s_src[m], s_dst[m], s_blk[m], s_hi[m]
        grp = cblk * 2 + chi  # non-decreasing (sorted)
        gcounts = np.bincount(grp, minlength=2 * cfg.NBLK)
        gstarts = np.zeros(2 * cfg.NBLK, np.int64)
        gstarts[1:] = np.cumsum(gcounts)[:-1]
        rank = np.arange(len(grp)) - gstarts[grp]
        slot = blk_off[cblk] + chi * (t_lo[cblk] * 128) + rank

        kv_val = np.where(chi == 1, csrc - cfg.HALF, csrc)
        kv_idx = np.zeros(slots, np.int64)
        kv_idx[slot] = kv_val
        qi_idx = np.zeros(slots, np.int64)
        qi_idx[slot] = cdst - c * cfg.SHARD

        S = np.zeros((128, slots), cfg.EDT_NP)
        scol = (slot // 128) * 128 + (cdst % 128)
        S[slot % 128, scol] = 1.0

        per_core.append(dict(kv_idx=_wrap16(kv_idx, slots),
                             qi_idx=_wrap16(qi_idx, slots), S=S))
    return per_core, t_lo.tolist(), t_hi.tolist(), blk_off.tolist(), slots


def _prep_inputs(cfg, inputs):
    x = np.asarray(inputs["x"], np.float32)
    idx = np.asarray(inputs["idx"]).astype(np.int64)
    f32 = lambda k: np.ascontiguousarray(np.asarray(inputs[k], np.float32))

    xp = np.zeros((cfg.N_PAD, cfg.F_IN), np.float32)
    xp[: cfg.N] = x

    per_core_e, t_lo, t_hi, blk_off, slots = _prep_edges(cfg, inputs["edge_index"])

    wkv1 = np.ascontiguousarray(
        np.concatenate([f32("c1_wk"), f32("c1_wv")], axis=1))     # [32,256]
    bkv1 = np.ascontiguousarray(
        np.concatenate([f32("c1_bk"), f32("c1_bv")])[None, :])    # [1,256]
    wkv2 = np.ascontiguousarray(
        np.concatenate([f32("c2_wk"), f32("c2_wv")], axis=1))     # [128,256]
    bkv2 = np.ascontiguousarray(
        np.concatenate([f32("c2_bk"), f32("c2_bv")])[None, :])
    qw1 = f32("q_w1")                                              # [288,128]
    bpad = ((cfg.B + 127) // 128) * 128

    in_maps = []
    for c in range(N_CORES):
        shard = slice(c * cfg.SHARD, (c + 1) * cfg.SHARD)
        own = (idx // cfg.SHARD) == c
        idx_loc = np.where(own, idx - c * cfg.SHARD, 0)
        im = dict(
            xT=np.ascontiguousarray(xp[shard].T),          # [128, SHARD]
            enc_w1=f32("enc_w1"),
            enc_b1=f32("enc_b1").reshape(32, 1),
            enc_w2=f32("enc_w2"),
            enc_b2c=f32("enc_b2").reshape(32, 1),
            enc_b2r=f32("enc_b2").reshape(1, 32),
            wq1=f32("c1_wq"), bq1=np.ascontiguousarray(f32("c1_bq")[None, :]),
            wkv1=wkv1, bkv1=bkv1,
            wq2=f32("c2_wq"), bq2=np.ascontiguousarray(f32("c2_bq")[None, :]),
            wkv2=wkv2, bkv2=bkv2,
            qw1a=np.ascontiguousarray(qw1[0:32]),
            qw1b=np.ascontiguousarray(qw1[32:160]),
            qw1c=np.ascontiguousarray(qw1[160:288]),
            qb1=f32("q_b1").reshape(128, 1),
            qw2=f32("q_w2"),
            qb2=f32("q_b2").reshape(1, 2),
            ones128=np.ones((1, 128), np.float32),
            id128=np.eye(128, dtype=np.float32),
            kv_idx=per_core_e[c]["kv_idx"],
            qi_idx=per_core_e[c]["qi_idx"],
            S_all=per_core_e[c]["S"],
            idx_x=_wrap16(idx_loc, bpad),
            own_mask=own.astype(np.float32).reshape(cfg.B, 1),
        )
        in_maps.append(im)
    return in_maps, t_lo, t_hi, blk_off, slots


# --------------------------------------------------------------------------
# device program
# --------------------------------------------------------------------------


def build_program(cfg, t_lo, t_hi, blk_off, slots):
    nc = bacc.Bacc("TRN2", target_bir_lowering=False, debug=False,
                   num_devices=N_CORES)
    EDT = cfg.EDT
    NB, SH = cfg.NBLK, cfg.SHARD
    RG = [list(range(N_CORES))]
    RELU = mybir.ActivationFunctionType.Relu
    COPY = mybir.ActivationFunctionType.Copy
    EXP = mybir.ActivationFunctionType.Exp

    def din(name, shape, dt=F32):
        return nc.dram_tensor(name, list(shape), dt, kind="ExternalInput").ap()

    xT = din("xT", [128, SH])
    enc_w1 = din("enc_w1", [128, 32]); enc_b1 = din("enc_b1", [32, 1])
    enc_w2 = din("enc_w2", [32, 32]); enc_b2c = din("enc_b2c", [32, 1])
    enc_b2r = din("enc_b2r", [1, 32])
    wq1 = din("wq1", [32, 128]); bq1 = din("bq1", [1, 128])
    wkv1 = din("wkv1", [32, 256]); bkv1 = din("bkv1", [1, 256])
    wq2 = din("wq2", [128, 128]); bq2 = din("bq2", [1, 128])
    wkv2 = din("wkv2", [128, 256]); bkv2 = din("bkv2", [1, 256])
    qw1a = din("qw1a", [32, 128]); qw1b = din("qw1b", [128, 128])
    qw1c = din("qw1c", [128, 128]); qb1 = din("qb1", [128, 1])
    qw2 = din("qw2", [128, 2]); qb2 = din("qb2", [1, 2])
    ones128 = din("ones128", [1, 128]); id128 = din("id128", [128, 128])
    kv_idx_d = din("kv_idx", [128, slots // 16], I16)
    qi_idx_d = din("qi_idx", [128, slots // 16], I16)
    S_d = din("S_all", [128, slots], EDT)
    bpad = ((cfg.B + 127) // 128) * 128
    idx_x_d = din("idx_x", [128, bpad // 16], I16)
    own_mask_d = din("own_mask", [cfg.B, 1])
    out_d = nc.dram_tensor("out", [cfg.B, 2], F32, kind="ExternalOutput").ap()

    with tile.TileContext(nc) as tc:
        with (
            tc.tile_pool(name="const", bufs=1) as cpool,
            tc.tile_pool(name="work", bufs=2) as wpool,
            tc.tile_pool(name="work1", bufs=1) as w1pool,
            tc.tile_pool(name="small", bufs=3) as spool,
            tc.tile_pool(name="psA", bufs=2, space="PSUM") as psA,
            tc.tile_pool(name="psB", bufs=2, space="PSUM") as psB,
            tc.tile_pool(name="psC", bufs=2, space="PSUM") as psC,
            tc.tile_pool(name="dram", bufs=1, space="DRAM") as dpool,
        ):
            nc.gpsimd.load_library(library_config.mlp)

            def ld(ap, shape, dt=F32, nm=None):
                t = cpool.tile(shape, dt, name=nm or ("ld_" + ap.tensor.name))
                nc.sync.dma_start(t[:], ap[:])
                return t

            w_enc1 = ld(enc_w1, [128, 32]); b_enc1 = ld(enc_b1, [32, 1])
            w_enc2 = ld(enc_w2, [32, 32]); b_enc2c = ld(enc_b2c, [32, 1])
            b_enc2r = ld(enc_b2r, [1, 32])
            w_q1 = ld(wq1, [32, 128]); b_q1 = ld(bq1, [1, 128])
            w_kv1 = ld(wkv1, [32, 256]); b_kv1 = ld(bkv1, [1, 256])
            w_q2 = ld(wq2, [128, 128]); b_q2 = ld(bq2, [1, 128])
            w_kv2 = ld(wkv2, [128, 256]); b_kv2 = ld(bkv2, [1, 256])
            w_qha = ld(qw1a, [32, 128]); w_qhb = ld(qw1b, [128, 128])
            w_qhc = ld(qw1c, [128, 128])
            b_qh = ld(qb1, [128, 1]); w_qh2 = ld(qw2, [128, 2])
            b_qh2 = ld(qb2, [1, 2])
            ones_s = ld(ones128, [1, 128]); id_s = ld(id128, [128, 128])
            nidx16 = (cfg.B + 15) // 16
            idxx_s = cpool.tile([128, nidx16], I16, name="idxx_s")
            nc.sync.dma_start(idxx_s[:], idx_x_d[:, 0:nidx16])
            mask_s = ld(own_mask_d, [cfg.B, 1])

            q1_tab = dpool.tile([SH, 128], EDT, name="q1_tab")
            q2_tab = dpool.tile([SH, 128], EDT, name="q2_tab")
            kv1_sh = dpool.tile([SH, 256], EDT, name="kv1_sh")
            kv2_sh = dpool.tile([SH, 256], EDT, name="kv2_sh")
            kv1_full = dpool.tile([cfg.N_PAD, 256], EDT, name="kv1_full",
                                  addr_space="Shared")
            kv2_full = dpool.tile([cfg.N_PAD, 256], EDT, name="kv2_full",
                                  addr_space="Shared")
            kv1_hi = dpool.tile([cfg.HALF, 256], EDT, name="kv1_hi")
            kv2_hi = dpool.tile([cfg.HALF, 256], EDT, name="kv2_hi")
            h0_rows = dpool.tile([SH, 64], F32, name="h0_rows")
            h1_rows = dpool.tile([SH, 128], F32, name="h1_rows")
            h2_rows = dpool.tile([SH, 128], F32, name="h2_rows")
            h1T_d = dpool.tile([128, SH], F32, name="h1T_d")
            ar_in = dpool.tile([cfg.B, 2], F32, name="ar_in")
            ar_out = dpool.tile([cfg.B, 2], F32, name="ar_out",
                                addr_space="Shared")

            # ============ encoder + conv1 tables, fused per block ============
            def emit_tab_block(hT_blk, bsl, w_q, b_q_, w_kv, b_kv_, q_tab,
                               kv_sh):
                psq = psA.tile([128, 128], F32, tag="psA", name="ps_q")
                nc.tensor.matmul(psq[:], hT_blk, w_q[:], start=True,
                                 stop=False)
                nc.tensor.matmul(psq[:], ones_s[:], b_q_[:],
                                 start=False, stop=True)
                qr = spool.tile([128, 128], EDT, tag="qr", name="qr")
                nc.scalar.activation(qr[:], psq[:], COPY)
                nc.sync.dma_start(q_tab[bsl, :], qr[:])

                psk = psB.tile([128, 256], F32, tag="psB", name="ps_kv")
                nc.tensor.matmul(psk[:], hT_blk, w_kv[:], start=True,
                                 stop=False)
                nc.tensor.matmul(psk[:], ones_s[:], b_kv_[:],
                                 start=False, stop=True)
                kvr = spool.tile([128, 256], EDT, tag="kvr", name="kvr")
                nc.vector.tensor_copy(kvr[:], psk[:])
                nc.sync.dma_start(kv_sh[bsl, :], kvr[:])

            for b in range(NB):
                bsl = slice(b * 128, (b + 1) * 128)
                xch = wpool.tile([128, 128], F32, tag="xch", name="xch")
                nc.sync.dma_start(xch[:], xT[:, bsl])
                ps1 = psA.tile([32, 128], F32, tag="psA", name="ps_enc1")
                nc.tensor.matmul(ps1[:], w_enc1[:], xch[:], start=True,
                                 stop=True)
                h1p = spool.tile([32, 128], F32, tag="h1p", name="h1p")
                nc.scalar.activation(h1p[:], ps1[:], RELU, bias=b_enc1[:],
                                     scale=1.0)
                ps2 = psA.tile([32, 128], F32, tag="psA", name="ps_enc2")
                nc.tensor.matmul(ps2[:], w_enc2[:], h1p[:], start=True,
                                 stop=True)
                h0b = spool.tile([32, 128], F32, tag="h0b", name="h0b")
                nc.scalar.activation(h0b[:], ps2[:], RELU, bias=b_enc2c[:],
                                     scale=1.0)
                # h0 rows (for the final x1 = h[idx] row gather)
                psr = psA.tile([128, 32], F32, tag="psA", name="ps_h0r")
                nc.tensor.matmul(psr[:], h1p[:], w_enc2[:], start=True,
                                 stop=False)
                nc.tensor.matmul(psr[:], ones_s[:], b_enc2r[:],
                                 start=False, stop=True)
                h0r = spool.tile([128, 64], F32, tag="h0r", name="h0r")
                nc.vector.memset(h0r[:, 32:64], 0.0)
                nc.scalar.activation(h0r[:, 0:32], psr[:], RELU)
                nc.sync.dma_start(h0_rows[bsl, :], h0r[:])
                # conv1 q/kv table rows
                emit_tab_block(h0b[:], bsl, w_q1, b_q1, w_kv1, b_kv1,
                               q1_tab, kv1_sh)
            nc.gpsimd.collective_compute(
                "AllGather", mybir.AluOpType.bypass, replica_groups=RG,
                ins=[kv1_sh.opt()], outs=[kv1_full.opt()])
            nc.sync.dma_start(kv1_hi[:, :], kv1_full[cfg.HALF: cfg.N_PAD, :])

            # ================= conv layers =================
            def emit_conv(q_tab, kv_full, kv_hi_tab, h_rows_out, h_T_out):
                for b in range(NB):
                    T_l, T_h = t_lo[b], t_hi[b]
                    T = T_l + T_h
                    off = blk_off[b]
                    oc = off // 16

                    # zero-offset idx tiles, one per gather
                    kvil = wpool.tile([128, T_l * 8], I16, tag="kvil",
                                      name="kvil")
                    nc.sync.dma_start(kvil[:], kv_idx_d[:, oc: oc + T_l * 8])
                    kvih = wpool.tile([128, T_h * 8], I16, tag="kvih",
                                      name="kvih")
                    nc.sync.dma_start(
                        kvih[:], kv_idx_d[:, oc + T_l * 8: oc + T * 8])
                    qil = wpool.tile([128, T_l * 8], I16, tag="qil",
                                     name="qil")
                    nc.sync.dma_start(qil[:], qi_idx_d[:, oc: oc + T_l * 8])
                    qih = wpool.tile([128, T_h * 8], I16, tag="qih",
                                     name="qih")
                    nc.sync.dma_start(
                        qih[:], qi_idx_d[:, oc + T_l * 8: oc + T * 8])
                    S_b = wpool.tile([128, T * 128], EDT, tag="S_b", name="S_b")
                    nc.sync.dma_start(S_b[:], S_d[:, off: off + T * 128])

                    # zero-offset gather destinations, one per gather
                    kv_lo = wpool.tile([128, T_l, 256], EDT, tag="kv_lo",
                                       name="kv_lo")
                    nc.gpsimd.dma_gather(
                        kv_lo[:], kv_full[0: cfg.HALF, :],
                        kvil[:], T_l * 128, T_l * 128, 256,
                        single_packet=False)
                    kv_hi = wpool.tile([128, T_h, 256], EDT, tag="kv_hi",
                                       name="kv_hi")
                    nc.gpsimd.dma_gather(
                        kv_hi[:], kv_hi_tab[:, :],
                        kvih[:], T_h * 128, T_h * 128, 256,
                        single_packet=False)
                    qi_lo = wpool.tile([128, T_l, 128], EDT, tag="qi_lo",
                                       name="qi_lo")
                    nc.gpsimd.dma_gather(
                        qi_lo[:], q_tab[:, :], qil[:], T_l * 128, T_l * 128,
                        128, single_packet=False)
                    qi_hi = wpool.tile([128, T_h, 128], EDT, tag="qi_hi",
                                       name="qi_hi")
                    nc.gpsimd.dma_gather(
                        qi_hi[:], q_tab[:, :], qih[:], T_h * 128, T_h * 128,
                        128, single_packet=False)

                    prod = w1pool.tile([128, T, 128], EDT, tag="prod",
                                       name="prod")
                    nc.vector.tensor_tensor(prod[:, 0:T_l, :],
                                            qi_lo[:],
                                            kv_lo[:, :, 0:128],
                                            mybir.AluOpType.mult)
                    nc.vector.tensor_tensor(prod[:, T_l:T, :],
                                            qi_hi[:],
                                            kv_hi[:, :, 0:128],
                                            mybir.AluOpType.mult)
                    l_t = spool.tile([128, T * 4], F32, tag="l_t", name="l_t")
                    nc.vector.tensor_reduce(
                        l_t[:].rearrange("p (t h) -> p t h", h=4),
                        prod[:].rearrange("p t (h j) -> p t h j", h=4, j=32),
                        mybir.AxisListType.X, mybir.AluOpType.add)

                    rhs = wpool.tile([128, T, 132], EDT, tag="rhs", name="rhs")
                    nc.scalar.activation(
                        rhs[:, :, 0:4],
                        l_t[:].rearrange("p (t h) -> p t h", h=4),
                        EXP, scale=float(cfg.SCALE))
                    aw = w1pool.tile([128, T, 128], EDT, tag="aw", name="aw")
                    nc.scalar.activation(
                        aw[:].rearrange("p t (h j) -> p t h j", h=4, j=32),
                        l_t[:].rearrange("p (t h) -> p t h", h=4)
                            .unsqueeze(-1).broadcast_to([128, T, 4, 32]),
                        EXP, scale=float(cfg.SCALE))
                    nc.vector.tensor_tensor(rhs[:, 0:T_l, 4:132],
                                            kv_lo[:, :, 128:256],
                                            aw[:, 0:T_l, :],
                                            mybir.AluOpType.mult)
                    nc.vector.tensor_tensor(rhs[:, T_l:T, 4:132],
                                            kv_hi[:, :, 128:256],
                                            aw[:, T_l:T, :],
                                            mybir.AluOpType.mult)

                    sc_ps = psB.tile([128, 132], F32, tag="psB", name="sc_ps")
                    for t in range(T):
                        nc.tensor.matmul(
                            sc_ps[:], S_b[:, t * 128: (t + 1) * 128],
                            rhs[:, t, :], start=(t == 0), stop=(t == T - 1))

                    den = spool.tile([128, 4], F32, tag="den", name="den")
                    nc.vector.tensor_scalar_add(den[:], sc_ps[:, 0:4],
                                                float(cfg.EPS))
                    rec = spool.tile([128, 4], F32, tag="rec", name="rec")
                    nc.vector.reciprocal(rec[:], den[:])
                    h_blk = spool.tile([128, 128], F32, tag="h_blk",
                                       name="h_blk")
                    for h in range(4):
                        nc.scalar.activation(
                            h_blk[:, h * 32: (h + 1) * 32],
                            sc_ps[:, 4 + h * 32: 4 + (h + 1) * 32],
                            RELU, scale=rec[:, h: h + 1])
                    bsl = slice(b * 128, (b + 1) * 128)
                    nc.sync.dma_start(h_rows_out[bsl, :], h_blk[:])
                    if h_T_out is not None:
                        tr_ps = psC.tile([128, 128], F32, tag="psC",
                                         name="tr_ps")
                        nc.tensor.transpose(tr_ps[:], h_blk[:], id_s[:])
                        h1tb = spool.tile([128, 128], F32, tag="h1tb",
                                          name="h1tb")
                        nc.scalar.activation(h1tb[:], tr_ps[:], COPY)
                        nc.sync.dma_start(h_T_out[:, bsl], h1tb[:])

            emit_conv(q1_tab, kv1_full, kv1_hi, h1_rows, h1T_d)

            # conv2 tables (h1T streamed back from DRAM per block)
            for b in range(NB):
                bsl = slice(b * 128, (b + 1) * 128)
                h1c = wpool.tile([128, 128], F32, tag="h1c", name="h1c")
                nc.sync.dma_start(h1c[:], h1T_d[:, bsl])
                emit_tab_block(h1c[:], bsl, w_q2, b_q2, w_kv2, b_kv2,
                               q2_tab, kv2_sh)
            nc.gpsimd.collective_compute(
                "AllGather", mybir.AluOpType.bypass, replica_groups=RG,
                ins=[kv2_sh.opt()], outs=[kv2_full.opt()])
            nc.sync.dma_start(kv2_hi[:, :], kv2_full[cfg.HALF: cfg.N_PAD, :])

            emit_conv(q2_tab, kv2_full, kv2_hi, h2_rows, None)

            # ================= Q head =================
            def gather_xT(tab, width):
                g = spool.tile([128, 1, width], F32, tag="gx", name="gx")
                nc.gpsimd.dma_gather(g[:], tab[:, :], idxx_s[:],
                                     cfg.B, cfg.B, width)
                tp = psC.tile([128, 128], F32, tag="psC", name="tp_x")
                nc.tensor.transpose(tp[0:width, 0: cfg.B], g[0: cfg.B, 0, :],
                                    id_s[0: cfg.B, 0: cfg.B])
                xt = spool.tile([128, cfg.B], F32, tag="xt", name="xt")
                nc.scalar.activation(xt[0:width, :], tp[0:width, 0: cfg.B],
                                     COPY)
                return xt

            x1t = gather_xT(h0_rows, 64)
            x2t = gather_xT(h1_rows, 128)
            x3t = gather_xT(h2_rows, 128)

            zh_ps = psA.tile([128, cfg.B], F32, tag="psA", name="zh_ps")
            nc.tensor.matmul(zh_ps[:], w_qha[:], x1t[0:32, :],
                             start=True, stop=False)
            nc.tensor.matmul(zh_ps[:], w_qhb[:], x2t[0:128, :],
                             start=False, stop=False)
            nc.tensor.matmul(zh_ps[:], w_qhc[:], x3t[0:128, :],
                             start=False, stop=True)
            zh = spool.tile([128, cfg.B], F32, tag="zh", name="zh")
            nc.scalar.activation(zh[:], zh_ps[:], RELU, bias=b_qh[:],
                                 scale=1.0)
            o_ps = psB.tile([cfg.B, 2], F32, tag="psB", name="o_ps")
            nc.tensor.matmul(o_ps[:], zh[:], w_qh2[:], start=True, stop=False)
            nc.tensor.matmul(o_ps[:], ones_s[:, 0: cfg.B], b_qh2[:],
                             start=False, stop=True)
            ob = spool.tile([cfg.B, 2], F32, tag="ob", name="ob")
            nc.vector.tensor_scalar_mul(ob[:], o_ps[:], mask_s[:])
            nc.sync.dma_start(ar_in[:, :], ob[:])
            nc.gpsimd.collective_compute(
                "AllReduce", mybir.AluOpType.add, replica_groups=RG,
                ins=[ar_in.opt()], outs=[ar_out.opt()])
            nc.sync.dma_start(out_d[:, :], ar_out[:, :])

    nc.compile()
    return nc


# --------------------------------------------------------------------------
# entry point
# --------------------------------------------------------------------------

_trace_flag = {"trace": False}
_last = {}


def _run(inputs, cfg=None):
    cfg = cfg or Cfg()
    in_maps, t_lo, t_hi, blk_off, slots = _prep_inputs(cfg, inputs)
    key = (slots, tuple(t_lo), tuple(t_hi), cfg.edge_bf16)
    if _last.get("key") != key:
        _last["nc"] = build_program(cfg, t_lo, t_hi, blk_off, slots)
        _last["key"] = key
    nc = _last["nc"]
    res = bass_utils.run_bass_kernel_spmd(
        nc, in_maps, core_ids=list(range(N_CORES)),
        trace=_trace_flag["trace"])
    _last["res"] = res
    return res.results[0]["out"].astype(np.float32)


def kernel(**inputs):
    return _run(inputs)

